# revision 5
# baseline (speedup 1.0000x reference)
"""MoE-INR fused Trainium2 kernel (8-core data parallel).

Layout: feature-major on chip (features on partitions, batch points on the
free dim). Each core processes B/8 = 65536 points in NT tiles of N points.

Math notes:
 - All SIREN omega scaling (x30) is folded into weights/biases host-side.
 - sin args are range-reduced: psA (policy layer reaches |z|~50) with a
   magic-number round (z - 2pi*round(z/2pi)); H1/H2/F2 (|z| <= 3pi) with the
   single-period add_range_wrap custom DVE op.
 - ACT Sin is accurate to ~|x|<=3.39 on HW, so s2's small bias (|b|<=0.23)
   is applied after the wrap inside the sin activation.
 - softmax's exp is rewritten via tanh (same ACT table set as sin):
   with q = tanh(-l/2), r = 1/(1+q): w = 2r - 1 = e^l exactly.
   y = sum_e w_e p_e / sum_e w_e   (softmax-weighted expert mix)
 - The 7-expert weighted sums are selector matmuls on the PE; the final
   division is a tiny [4, n/2] reciprocal+multiply per 4-tile block.
"""
import sys

sys.path.insert(0, "/opt/trn_rl_repo")

import numpy as np

import concourse.bass as bass
import concourse.mybir as mybir
import concourse.tile as tile
from concourse import bacc, bass_utils

AF = mybir.ActivationFunctionType
ALU = mybir.AluOpType
F32 = mybir.dt.float32

PI = float(np.pi)
TWO_PI = float(2 * np.pi)
MAGIC = 12582912.0  # 1.5 * 2^23: forces round-to-nearest-integer in fp32
OMEGA = 30.0

B = 524288
IN = 4
BD = 32
ENC = 8 * BD  # 256
E = 7
NCORES = 8
PTS = B // NCORES  # 65536
N = 1024  # points per tile
NT = PTS // N  # 64 tiles

# engine assignment knobs (tuned empirically)
ENG_T12_RELU = "vector"
ENG_ENCA_RELU = "scalar"
ENG_ENCB_RELU = "vector"
ENG_SB = "gpsimd"
ENG_WB = "gpsimd"
ENG_MB = "vector"
ENG_K = "gpsimd"
ENG_PCOPY = "scalar"


def _prep_weights(w):
    """Host-side preprocessing of the reference weights into device tensors."""
    f32 = np.float32

    def T(a):
        return np.ascontiguousarray(np.asarray(a, f32))

    pe_w, pe_b = T(w["pe_w"]), T(w["pe_b"])
    s1_w, s1_b = T(w["s1_w"]), T(w["s1_b"])
    s2_w, s2_b = T(w["s2_w"]), T(w["s2_b"])
    fc1_w, fc1_b = T(w["fc1_w"]), T(w["fc1_b"])
    fc2_w, fc2_b = T(w["fc2_w"]), T(w["fc2_b"])
    fc3_w, fc3_b = T(w["fc3_w"]), T(w["fc3_b"])
    p1_w, p1_b = T(w["p1_w"]), T(w["p1_b"])
    p2_w, p2_b = T(w["p2_w"]), T(w["p2_b"])
    gate_w, gate_b = T(w["gate_w"]), T(w["gate_b"])
    exp_w, exp_b = T(w["exp_w"]), T(w["exp_b"])

    t = {}
    # W_A [5, 128]: psA rows = 0 pi/2 | 1-32 zp1 | 33-64 u_sin | 65-96 u_cos | 97+ zero
    WA = np.zeros((5, 128), f32)
    WA[4, 0] = PI / 2
    WA[0:4, 1:33] = OMEGA * p1_w.T
    WA[4, 1:33] = OMEGA * p1_b
    WA[0:4, 33:65] = pe_w.T
    WA[4, 33:65] = pe_b
    WA[0:4, 65:97] = pe_w.T
    WA[4, 65:97] = pe_b + PI / 2
    t["WA"] = WA

    # SA rows: 0 ones | 1-32 f1 | 33-96 h0=[sin u, cos u] | 97-127 junk
    # s1 lhsT rows 0..96 (row 0 = bias via ones row; rows 1-32 zero-skip f1)
    Ws1 = np.zeros((128, 128), f32)
    Ws1[0, :] = OMEGA * s1_b
    Ws1[33:97, :] = OMEGA * s1_w.T
    t["Ws1"] = Ws1

    # p2 lhsT rows 0..32 ([ones; f1])
    Wp2 = np.zeros((128, 32), f32)
    Wp2[0, :] = OMEGA * p2_b
    Wp2[1:33, :] = OMEGA * p2_w.T
    t["Wp2"] = Wp2

    t["Ws2a"] = T(OMEGA * s2_w.T[:, 0:128])
    t["Ws2b"] = T(OMEGA * s2_w.T[:, 128:256])
    t["bias_h2a"] = T(OMEGA * s2_b[0:128]).reshape(128, 1)
    t["bias_h2b"] = T(OMEGA * s2_b[128:256]).reshape(128, 1)

    t["Wfc1a"] = T(fc1_w.T[0:128, :])  # [128, 64]
    t["Wfc1b"] = T(fc1_w.T[128:256, :])
    Wfc2 = np.zeros((128, 64), f32)  # at rows 64-127 (rhs = t1 rows 64-127)
    Wfc2[64:128, :] = fc2_w.T
    t["Wfc2"] = Wfc2
    t["Wfc3a"] = T(fc3_w.T[:, 0:128])  # [64, 128] at rows 0-63
    t["Wfc3b"] = T(fc3_w.T[:, 128:256])
    bias_t12 = np.zeros((128, 1), f32)
    bias_t12[0:64, 0] = fc2_b
    bias_t12[64:128, 0] = fc1_b
    t["bias_t12"] = bias_t12
    t["bias_enca"] = T(fc3_b[0:128]).reshape(128, 1)
    t["bias_encb"] = T(fc3_b[128:256]).reshape(128, 1)
    t["I128"] = np.eye(128, dtype=f32)

    # gate enc halves [128, 32] (cols 0-6 hot, rest zero -> psum rows zero)
    ga = np.zeros((128, 32), f32)
    gb = np.zeros((128, 32), f32)
    ga[:, 0:E] = gate_w[:, 0:128].T
    gb[:, 0:E] = gate_w[:, 128:256].T
    t["Wga"] = ga
    t["Wgb"] = gb
    gf2 = np.zeros((128, 32), f32)  # f2 part replicated at 4 row offsets
    for j in range(4):
        gf2[32 * j : 32 * j + 32, 0:E] = gate_w[:, 256:288].T
    t["Wgf2"] = gf2
    pw = exp_w.reshape(E, ENC).T  # [256, 7]
    pa = np.zeros((128, 32), f32)
    pb_ = np.zeros((128, 32), f32)
    pa[:, 0:E] = pw[0:128, :]
    pb_[:, 0:E] = pw[128:256, :]
    t["Wpa"] = pa
    t["Wpb"] = pb_
    bias_pred = np.zeros((128, 1), f32)
    bias_tanh = np.zeros((128, 1), f32)
    for j in range(4):
        bias_pred[32 * j : 32 * j + E, 0] = exp_b[:, 0]
        bias_tanh[32 * j : 32 * j + E, 0] = -0.5 * gate_b
    t["bias_pred"] = bias_pred
    t["bias_tanh"] = bias_tanh

    # selector matmul: col j sums rows 32j+e (e<7); used for both num and den
    selN = np.zeros((128, 128), f32)
    for j in range(4):
        for e in range(E):
            selN[32 * j + e, j] = 1.0
    t["selN"] = selN

    t["bias_zero"] = np.zeros((128, 1), f32)
    return t


WSPEC = {
    "WA": (5, 128), "Ws1": (128, 128), "Wp2": (128, 32),
    "Ws2a": (128, 128), "Ws2b": (128, 128),
    "Wfc1a": (128, 64), "Wfc1b": (128, 64), "Wfc2": (128, 64),
    "Wfc3a": (64, 128), "Wfc3b": (64, 128), "I128": (128, 128),
    "Wga": (128, 32), "Wgb": (128, 32), "Wgf2": (128, 32),
    "Wpa": (128, 32), "Wpb": (128, 32),
    "selN": (128, 128),
    "bias_h2a": (128, 1), "bias_h2b": (128, 1), "bias_t12": (128, 1),
    "bias_enca": (128, 1), "bias_encb": (128, 1),
    "bias_pred": (128, 1), "bias_tanh": (128, 1),
    "bias_zero": (128, 1),
}


def _eng(nc, name):
    return {"vector": nc.vector, "gpsimd": nc.gpsimd, "scalar": nc.scalar}[name]


def build_program(pts=PTS, n=N, dbg=False):
    """Build + compile the per-core Bass program."""
    nt = pts // n
    nh = n // 2
    assert nt % 4 == 0

    nc = bacc.Bacc("TRN2", target_bir_lowering=False, debug=False, num_devices=1)

    xt_d = nc.dram_tensor("xt", (5, pts), F32, kind="ExternalInput").ap()
    y_d = nc.dram_tensor("y", (pts,), F32, kind="ExternalOutput").ap()
    wd = {k: nc.dram_tensor("w_" + k, shp, F32, kind="ExternalInput").ap()
          for k, shp in WSPEC.items()}
    dbg_names = ["sa0", "sh1_0", "sh2a0", "sh2b0", "st12_1", "sea0", "seb0",
                 "f2s0", "qb0", "pb0", "wb0", "mb0", "sel0"]
    dbg_d = {}
    if dbg:
        for k in dbg_names:
            shp = (128, 512) if k in ("qb0", "pb0", "wb0", "mb0", "sel0") else (128, n)
            dbg_d[k] = nc.dram_tensor("dbg_" + k, shp, F32, kind="ExternalOutput").ap()

    def dump(name, tileap):
        if dbg and name in dbg_d:
            nc.gpsimd.dma_start(dbg_d[name][:], tileap)

    with tile.TileContext(nc) as tc:
        with (
            tc.tile_pool(name="const", bufs=1) as cpool,
            tc.tile_pool(name="xt", bufs=3) as xpool,
            tc.tile_pool(name="sa", bufs=6) as sapool,
            tc.tile_pool(name="sh1", bufs=2) as h1pool,
            tc.tile_pool(name="sh2a", bufs=3) as h2apool,
            tc.tile_pool(name="sh2b", bufs=3) as h2bpool,
            tc.tile_pool(name="st12", bufs=3) as t12pool,
            tc.tile_pool(name="sea", bufs=6) as seapool,
            tc.tile_pool(name="seb", bufs=6) as sebpool,
            tc.tile_pool(name="f2s", bufs=2) as f2pool,
            tc.tile_pool(name="tmp", bufs=2) as tmppool,
            tc.tile_pool(name="tail", bufs=2) as tailpool,
            tc.tile_pool(name="psbody", bufs=2, space="PSUM") as psbody,
            tc.tile_pool(name="pst", bufs=2, space="PSUM") as pst,
            tc.tile_pool(name="pslp", bufs=2, space="PSUM") as pslp,
        ):
            W = {}
            for k, shp in WSPEC.items():
                wt = cpool.tile(list(shp), F32, tag="w_" + k)
                nc.gpsimd.dma_start(wt[:], wd[k][:])
                W[k] = wt

            xr = xt_d.rearrange("f (t n) -> f t n", n=n)  # [5, nt, n]
            yr = y_d.rearrange("(t n) -> t n", n=n)  # [nt, n]

            st12_by_t = {}  # t -> ST12 tile ([t2(t-1); t1(t)])
            sh2_hist = {}  # t -> (SH2a, SH2b)
            sa_hist = {}  # t -> SA
            se_hist = {}  # v -> (SEa, SEb)
            f2_hist = {}  # block -> F2S

            def mm(out, lhsT, rhs, start, stop, tp=None):
                if tp is None:
                    nc.tensor.matmul(out, lhsT, rhs, start=start, stop=stop)
                else:
                    nc.tensor.matmul(out, lhsT, rhs, start=start, stop=stop,
                                     tile_position=tp)

            for t in range(nt + 1):
                u = t
                v = t - 1

                if u < nt:
                    # ---------- FRONT(u) ----------
                    xt = xpool.tile([5, n], F32, tag="xt")
                    nc.gpsimd.dma_start(xt[:], xr[:, u, :])

                    psA = psbody.tile([128, n], F32, tag="body")
                    for h in range(2):
                        mm(psA[:, h * nh:(h + 1) * nh], W["WA"][:],
                           xt[:, h * nh:(h + 1) * nh], True, True)
                    # magic range-reduce all 128 rows: z -= 2pi*round(z/2pi)
                    jt = tmppool.tile([128, n], F32, tag="jt")
                    nc.vector.tensor_scalar(
                        jt[:], psA[:], float(1.0 / TWO_PI), MAGIC, ALU.mult, ALU.add)
                    kt = tmppool.tile([128, n], F32, tag="kt")
                    _eng(nc, ENG_K).tensor_scalar(kt[:], jt[:], MAGIC, None, ALU.subtract)
                    nc.vector.scalar_tensor_tensor(
                        psA[:], in0=kt[:], scalar=-TWO_PI, in1=psA[:],
                        op0=ALU.mult, op1=ALU.add)
                    sa = sapool.tile([128, n], F32, tag="sa")
                    nc.scalar.activation(sa[:], psA[:], AF.Sin, bias=W["bias_zero"][:])
                    sa_hist[u] = sa
                    if u == 0:
                        dump("sa0", sa[:])

                    # H1
                    psH1 = psbody.tile([128, n], F32, tag="body")
                    for h in range(2):
                        mm(psH1[:, h * nh:(h + 1) * nh], W["Ws1"][0:97, :],
                           sa[0:97, h * nh:(h + 1) * nh], True, True)
                    nc.vector.add_range_wrap(psH1[:], psH1[:], shift=0.0,
                                             bound=PI, period=TWO_PI)
                    sh1 = h1pool.tile([128, n], F32, tag="sh1")
                    nc.scalar.activation(sh1[:], psH1[:], AF.Sin, bias=W["bias_zero"][:])
                    if u == 0:
                        dump("sh1_0", sh1[:])

                    # H2a / H2b (bias after wrap, |b|<=0.23)
                    sh2 = []
                    for half, (wk, bk, pool) in enumerate(
                        [("Ws2a", "bias_h2a", h2apool), ("Ws2b", "bias_h2b", h2bpool)]
                    ):
                        psH2 = psbody.tile([128, n], F32, tag="body")
                        for h in range(2):
                            mm(psH2[:, h * nh:(h + 1) * nh], W[wk][:],
                               sh1[:, h * nh:(h + 1) * nh], True, True)
                        nc.vector.add_range_wrap(psH2[:], psH2[:], shift=0.0,
                                                 bound=PI, period=TWO_PI)
                        s = pool.tile([128, n], F32, tag="sh2_%d" % half)
                        nc.scalar.activation(s[:], psH2[:], AF.Sin, bias=W[bk][:])
                        if u == 0:
                            dump("sh2a0" if half == 0 else "sh2b0", s[:])
                        sh2.append(s)
                    sh2_hist[u] = tuple(sh2)

                    # deferred p2 matmuls + f2 sin at end of each 4-block
                    if u % 4 == 3:
                        bsel = u // 4
                        psF2 = psbody.tile([128, n], F32, tag="body")
                        for j in range(4):
                            saj = sa_hist.pop(4 * bsel + j)
                            for h in range(2):
                                mm(psF2[32 * j:32 * j + 32, h * nh:(h + 1) * nh],
                                   W["Wp2"][0:33, :], saj[0:33, h * nh:(h + 1) * nh],
                                   True, True, tp=(0, 32 * j))
                        nc.vector.add_range_wrap(psF2[:], psF2[:], shift=0.0,
                                                 bound=PI, period=TWO_PI)
                        f2s = f2pool.tile([128, n], F32, tag="f2s")
                        nc.scalar.activation(f2s[:], psF2[:], AF.Sin,
                                             bias=W["bias_zero"][:])
                        if bsel == 0:
                            dump("f2s0", f2s[:])
                        f2_hist[bsel] = f2s

                    # fc1 -> psT12 rows 64-127; fc2(t-1) -> rows 0-63; relu
                    psT12h = []
                    for h in range(2):
                        ps = pst.tile([128, nh], F32, tag="pst")
                        mm(ps[64:128, :], W["Wfc1a"][:],
                           sh2[0][:, h * nh:(h + 1) * nh], True, False, tp=(0, 64))
                        mm(ps[64:128, :], W["Wfc1b"][:],
                           sh2[1][:, h * nh:(h + 1) * nh], False, True, tp=(0, 64))
                        psT12h.append(ps)
                    st12 = t12pool.tile([128, n], F32, tag="st12")
                    eng = _eng(nc, ENG_T12_RELU)
                    if t >= 1:
                        prev = st12_by_t[t - 1]
                        for h in range(2):
                            mm(psT12h[h][0:64, :], W["Wfc2"][64:128, :],
                               prev[64:128, h * nh:(h + 1) * nh], True, True, tp=(64, 0))
                            eng.tensor_scalar(
                                st12[:, h * nh:(h + 1) * nh], psT12h[h][:],
                                W["bias_t12"][:], 0.0, ALU.add, ALU.max)
                    else:
                        for h in range(2):
                            eng.tensor_scalar(
                                st12[64:128, h * nh:(h + 1) * nh], psT12h[h][64:128, :],
                                W["bias_t12"][64:128, :], 0.0, ALU.add, ALU.max)
                    if t == 1:
                        dump("st12_1", st12[:])
                    st12_by_t[t] = st12
                else:
                    # final tile: only fc2(nt-1) + relu rows 0-63
                    prev = st12_by_t[t - 1]
                    st12 = t12pool.tile([128, n], F32, tag="st12")
                    eng = _eng(nc, ENG_T12_RELU)
                    for h in range(2):
                        ps = pst.tile([128, nh], F32, tag="pst")
                        mm(ps[0:64, :], W["Wfc2"][64:128, :],
                           prev[64:128, h * nh:(h + 1) * nh], True, True, tp=(64, 0))
                        eng.tensor_scalar(
                            st12[0:64, h * nh:(h + 1) * nh], ps[0:64, :],
                            W["bias_t12"][0:64, :], 0.0, ALU.add, ALU.max)
                    st12_by_t[t] = st12

                if 0 <= v < nt:
                    # ---------- LATE(v): fc3 + enc ----------
                    st12v = st12_by_t[v + 1]  # rows 0-63 = t2(v)
                    if v - 1 in st12_by_t:
                        del st12_by_t[v - 1]
                    sh2a_v, sh2b_v = sh2_hist.pop(v)
                    ses = []
                    for half, (wk, bk, pool, eng_name, src) in enumerate([
                        ("Wfc3a", "bias_enca", seapool, ENG_ENCA_RELU, sh2a_v),
                        ("Wfc3b", "bias_encb", sebpool, ENG_ENCB_RELU, sh2b_v),
                    ]):
                        se = pool.tile([128, n], F32, tag="se_%d" % half)
                        for h in range(2):
                            psT3 = pst.tile([128, nh], F32, tag="pst")
                            mm(psT3[:], W[wk][:], st12v[0:64, h * nh:(h + 1) * nh],
                               True, False)
                            mm(psT3[:], W["I128"][:], src[:, h * nh:(h + 1) * nh],
                               False, True)
                            if eng_name == "scalar":
                                nc.scalar.activation(
                                    se[:, h * nh:(h + 1) * nh], psT3[:], AF.Relu,
                                    bias=W[bk][:])
                            else:
                                _eng(nc, eng_name).tensor_scalar(
                                    se[:, h * nh:(h + 1) * nh], psT3[:],
                                    W[bk][:], 0.0, ALU.add, ALU.max)
                        if v == 0:
                            dump("sea0" if half == 0 else "seb0", se[:])
                        ses.append(se)
                    se_hist[v] = tuple(ses)

                    # ---------- BLOCK(b) at v = 4b+3: gate/preds + tail ----------
                    if v % 4 == 3:
                        bidx = v // 4
                        f2s = f2_hist.pop(bidx)
                        for h in range(2):
                            psL = pslp.tile([128, nh], F32, tag="lp")
                            psP = pslp.tile([128, nh], F32, tag="lp")
                            for j in range(4):
                                sea_j, seb_j = se_hist[4 * bidx + j]
                                sl = psL[32 * j:32 * j + 32, :]
                                mm(sl, W["Wga"][:], sea_j[:, h * nh:(h + 1) * nh],
                                   True, False, tp=(0, 32 * j))
                                mm(sl, W["Wgb"][:], seb_j[:, h * nh:(h + 1) * nh],
                                   False, False, tp=(0, 32 * j))
                                mm(sl, W["Wgf2"][32 * j:32 * j + 32, :],
                                   f2s[32 * j:32 * j + 32, h * nh:(h + 1) * nh],
                                   False, True, tp=(32 * j, 32 * j))
                                sp = psP[32 * j:32 * j + 32, :]
                                mm(sp, W["Wpa"][:], sea_j[:, h * nh:(h + 1) * nh],
                                   True, False, tp=(0, 32 * j))
                                mm(sp, W["Wpb"][:], seb_j[:, h * nh:(h + 1) * nh],
                                   False, True, tp=(0, 32 * j))

                            qb = tailpool.tile([128, nh], F32, tag="qb")
                            nc.scalar.activation(qb[:], psL[:], AF.Tanh,
                                                 bias=W["bias_tanh"][:], scale=-0.5)
                            pb = tailpool.tile([128, nh], F32, tag="pb")
                            if ENG_PCOPY == "scalar":
                                nc.scalar.activation(pb[:], psP[:], AF.Identity,
                                                     bias=W["bias_pred"][:])
                            else:
                                _eng(nc, ENG_PCOPY).tensor_scalar(
                                    pb[:], psP[:], W["bias_pred"][:], None, ALU.add)
                            sb = tailpool.tile([128, nh], F32, tag="sb")
                            _eng(nc, ENG_SB).tensor_scalar(sb[:], qb[:], 1.0, None, ALU.add)
                            rb = tailpool.tile([128, nh], F32, tag="rb")
                            nc.vector.reciprocal_approx_fast(out=rb[:], in_=sb[:])
                            wb = tailpool.tile([128, nh], F32, tag="wb")
                            _eng(nc, ENG_WB).tensor_scalar(wb[:], rb[:], 2.0, -1.0,
                                                           ALU.mult, ALU.add)
                            mb = tailpool.tile([128, nh], F32, tag="mb")
                            _eng(nc, ENG_MB).tensor_tensor(mb[:], wb[:], pb[:], ALU.mult)
                            if bidx == 0 and h == 0:
                                dump("qb0", qb[:])
                                dump("pb0", pb[:])
                                dump("wb0", wb[:])
                                dump("mb0", mb[:])
                            psSelN = pslp.tile([128, nh], F32, tag="lp")
                            mm(psSelN[:], W["selN"][:], mb[:], True, True)
                            psSelD = pslp.tile([128, nh], F32, tag="lp")
                            mm(psSelD[:], W["selN"][:], wb[:], True, True)
                            if bidx == 0 and h == 0:
                                selcp = tailpool.tile([128, nh], F32, tag="selcp")
                                nc.vector.tensor_copy(selcp[0:4, :], psSelN[0:4, :])
                                nc.vector.tensor_copy(selcp[64:68, :], psSelD[0:4, :])
                                dump("sel0", selcp[:])
                            rden = tailpool.tile([4, nh], F32, tag="rden")
                            nc.vector.reciprocal_approx_fast(
                                out=rden[:], in_=psSelD[0:4, :])
                            yv = tailpool.tile([4, nh], F32, tag="yv")
                            nc.vector.tensor_tensor(yv[:], psSelN[0:4, :],
                                                    rden[:], ALU.mult)
                            nc.gpsimd.dma_start(
                                yr[4 * bidx:4 * bidx + 4, h * nh:(h + 1) * nh], yv[:])
                        for j in range(4):
                            del se_hist[4 * bidx + j]

    nc.compile()
    return nc


_CACHE = {}


def _get_program():
    if "nc" not in _CACHE:
        _CACHE["nc"] = build_program()
    return _CACHE["nc"]


def _make_in_maps(inputs):
    x = np.asarray(inputs["x"], np.float32)
    t = _prep_weights(inputs)
    xT = np.empty((5, B), np.float32)
    xT[0:4] = x.T
    xT[4] = 1.0
    in_maps = []
    for c in range(NCORES):
        m = {"xt": np.ascontiguousarray(xT[:, c * PTS:(c + 1) * PTS])}
        for k, val in t.items():
            m["w_" + k] = val
        in_maps.append(m)
    return in_maps


def kernel(**inputs) -> np.ndarray:
    in_maps = _make_in_maps(inputs)
    nc = _get_program()
    res = bass_utils.run_bass_kernel_spmd(nc, in_maps, core_ids=list(range(NCORES)))
    y = np.concatenate([res.results[c]["y"] for c in range(NCORES)])
    return y.reshape(B, 1)


if __name__ == "__main__":
    print("building program...")
    build_program()
    print("compiled ok")
